# revision 10
# baseline (speedup 1.0000x reference)
"""Causal self-attention (B=4, T=2048, C=1024, H=16) on 8 TRN2 NeuronCores.

Sharding: core c -> (batch b = c//2, head-group g = c%2 covering 8 heads).
Each core computes q,k,v + attention for its 8 heads of its batch, and the
output projection restricted to its group's 512 contraction dims; the two
partial [C, T] outputs per batch are summed on the host.

Device-side layouts are all "transposed" so no on-device transposes exist:
  xt  [C, T]   = x[b].T          (bf16, host-prepped)
  qT/kT [CG, T] via matmul(lhsT=W.T slice, rhs=xt)
  v   [T, CG]  via matmul(lhsT=xt, rhs=Wv.T slice), stored interleaved with a
               ones column per head ([v_h | 1] width 65) so the attention-value
               matmul also produces the softmax denominator in PSUM row 64.
  S_T [k, q]   per head via matmul(lhsT=kT chunk, rhs=qT chunk)  (K=hd=64)
  P_T = exp(S_T/8 + key_mask_bias) on ScalarE (bias is a per-partition [128,1]
        column: 0 for visible keys, -60 for masked -> weight ~e-60 ~ 0)
  causal: k-tile/q-chunk blocks above the diagonal are skipped entirely;
        diagonal 128x128 blocks get one triu-mask multiply.
  yT  = (1/denom * m_q) broadcast-multiplied into the AV PSUM on evacuation.
  out.T [C, T] = matmul(lhsT=Wp.T slice, rhs=yT) + rank-1 fixup
        r * (1 - m_q)  (r = (mean of all v rows) @ Wp.T) which reproduces the
        reference's uniform-attention behavior for fully-masked query rows.
"""

import numpy as np
import ml_dtypes

P = 128
HD = 64
VW = HD + 1
NEG_BIAS = -60.0
BF = ml_dtypes.bfloat16


class Dims:
    def __init__(self, B=4, T=2048, C=1024, H=16, n_cores=8, qc_max=1024):
        self.B, self.T, self.C, self.H, self.n_cores = B, T, C, H, n_cores
        self.gpb = n_cores // B          # head groups per batch (2)
        self.HPC = H // self.gpb         # heads per core (8)
        self.CG = self.HPC * HD          # group width (512)
        self.KT = C // P                 # contraction tiles for qkv (8)
        self.CT = C // P                 # output row tiles (8)
        self.TT = T // P                 # sequence tiles (16)
        self.MT = self.CG // P           # qT/kT row tiles = head pairs (4)
        self.JT = self.CG // P           # proj contraction tiles (4)
        self.QC = min(qc_max, T)         # attention q-chunk
        self.NQC = T // self.QC
        self.QCKT = self.QC // P         # k-tiles per q-chunk (8)
        self.HALF = min(512, self.QC)    # AV / normalize sub-chunk
        self.NHALF = self.QC // self.HALF
        self.HPP = self.HALF // P
        self.EV = min(512, self.QC)      # qkv/proj evac chunk
        self.CC = min(512, C)            # r_row chunk
        self.NCC = C // self.CC
        assert C % P == 0 and T % P == 0 and self.CG % P == 0
        assert self.CG <= 512


def build_program(d: Dims, pt_bufs=6):
    import concourse.mybir as mybir
    import concourse.tile as tile
    from concourse import bacc
    from contextlib import ExitStack

    F32, BF16 = mybir.dt.float32, mybir.dt.bfloat16
    Exp = mybir.ActivationFunctionType.Exp
    Ident = mybir.ActivationFunctionType.Identity
    ADD = mybir.AluOpType.add

    nc = bacc.Bacc("TRN2", target_bir_lowering=False, debug=False,
                   num_devices=d.n_cores)

    def inp(n, s, dt):
        return nc.dram_tensor(n, s, dt, kind="ExternalInput").ap()

    xt_d = inp("xt", [d.C, d.T], BF16)
    wqt_d = inp("wqt", [d.C, d.CG], BF16)
    wkt_d = inp("wkt", [d.C, d.CG], BF16)
    wvt_d = inp("wvt", [d.C, d.CG], BF16)
    wpt_d = inp("wpt", [d.CG, d.C], BF16)
    qb_d = inp("qb", [P, d.MT], F32)
    kb_d = inp("kb", [P, d.MT], F32)
    vb_d = inp("vb", [P, d.HPC * VW], F32)
    kbias_d = inp("kbias", [P, d.TT], F32)
    bpc_d = inp("bpc", [P, d.CT], F32)
    mq_d = inp("mq", [1, d.T], F32)
    omq_d = inp("omq", [1, d.T], BF16)
    triu_d = inp("triu", [P, P], BF16)
    ot_d = nc.dram_tensor("ot", [d.C, d.T], F32, kind="ExternalOutput").ap()
    ot_v = ot_d.rearrange("(c p) t -> c p t", p=P)

    with tile.TileContext(nc) as tc, ExitStack() as stk:
        const = stk.enter_context(tc.tile_pool(name="const", bufs=1))
        persist = stk.enter_context(tc.tile_pool(name="persist", bufs=1))
        psum = stk.enter_context(tc.tile_pool(name="psum", bufs=2, space="PSUM"))
        work = stk.enter_context(tc.tile_pool(name="work", bufs=2))

        TH = d.T // d.NQC            # columns per half (== QC)
        TTH = d.TT // d.NQC          # seq tiles per half

        # ---- small constants first (DMA order = issue order) ----
        qb_sb = const.tile([P, d.MT], F32)
        kb_sb = const.tile([P, d.MT], F32)
        vb_sb = const.tile([P, d.HPC * VW], F32)
        kbias_sb = const.tile([P, d.TT], F32)
        bpc_sb = const.tile([P, d.CT], F32)
        mq_sb = const.tile([1, d.T], F32)
        omq_sb = const.tile([1, d.T], BF16)
        triu_sb = const.tile([P, P], BF16)
        for s_, dd in ((qb_sb, qb_d), (kb_sb, kb_d), (vb_sb, vb_d),
                       (kbias_sb, kbias_d), (bpc_sb, bpc_d), (mq_sb, mq_d),
                       (omq_sb, omq_d), (triu_sb, triu_d)):
            nc.sync.dma_start(s_[:], dd[:])
        invT = const.tile([P, 1], BF16)
        nc.vector.memset(invT[:], 1.0 / d.T)

        # ---- weights: q/k/v now, wpt later ----
        wqt_sb = const.tile([P, d.KT, d.CG], BF16)
        wkt_sb = const.tile([P, d.KT, d.CG], BF16)
        wvt_sb = const.tile([P, d.KT, d.CG], BF16)
        wpt_sb = const.tile([P, d.JT, d.C], BF16)
        for w_sb, w_d in ((wqt_sb, wqt_d), (wkt_sb, wkt_d), (wvt_sb, wvt_d)):
            wv = w_d.rearrange("(k p) g -> k p g", p=P)
            for kt in range(d.KT):
                nc.sync.dma_start(w_sb[:, kt, :], wv[kt])

        # ---- per-half tensors (fine-grained deps across phases) ----
        xt_sbs = [None] * d.NQC
        qt_sbs = [persist.tile([P, d.MT, TH], BF16, name=f"qt{i}")
                  for i in range(d.NQC)]
        kt_sbs = [persist.tile([P, d.MT, TH], BF16, name=f"kt{i}")
                  for i in range(d.NQC)]
        v_sbs = [persist.tile([P, TTH, d.HPC * VW], BF16, name=f"v{i}")
                 for i in range(d.NQC)]
        yt_sbs = [persist.tile([P, d.JT, TH], BF16, name=f"yt{i}")
                  for i in range(d.NQC)]
        vm_sb = persist.tile([P, d.JT], BF16)
        r_sb = persist.tile([1, d.C], BF16)
        xv = xt_d.rearrange("(k p) t -> k p t", p=P)

        def qkv_chunks(hf):
            """Emit xt DMAs for half `hf` now; return closures for the
            matmul work, to be interleaved into other phases."""
            xt_sb = persist.tile([P, d.KT, TH], BF16, name=f"xt{hf}")
            xt_sbs[hf] = xt_sb
            nev = TH // d.EV
            for qn in range(nev):
                for kt in range(d.KT):
                    nc.sync.dma_start(
                        xt_sb[:, kt, qn * d.EV:(qn + 1) * d.EV],
                        xv[kt][:, hf * TH + qn * d.EV:
                               hf * TH + (qn + 1) * d.EV])
            chunks = []

            def qk_chunk(mt, qn):
                sl = slice(qn * d.EV, (qn + 1) * d.EV)
                for w_sb, o_sb, b_sb in ((wkt_sb, kt_sbs[hf], kb_sb),
                                         (wqt_sb, qt_sbs[hf], qb_sb)):
                    ps = psum.tile([P, d.EV], F32, tag="acc", bufs=4,
                                   name=f"qk{hf}_{mt}_{qn}_{w_sb.name}")
                    for kt in range(d.KT):
                        nc.tensor.matmul(
                            ps[:], w_sb[:, kt, mt * P:(mt + 1) * P],
                            xt_sb[:, kt, sl],
                            start=(kt == 0), stop=(kt == d.KT - 1))
                    nc.vector.tensor_scalar_add(
                        o_sb[:, mt, sl], ps[:], b_sb[:, mt:mt + 1])

            def v_chunk(tt):
                ps = psum.tile([P, d.CG], F32, tag="acc", bufs=4,
                               name=f"vps{hf}_{tt}")
                for kt in range(d.KT):
                    nc.tensor.matmul(
                        ps[:], xt_sb[:, kt, tt * P:(tt + 1) * P],
                        wvt_sb[:, kt, :],
                        start=(kt == 0), stop=(kt == d.KT - 1))
                vv = v_sbs[hf][:, tt, :].rearrange("p (h w) -> p h w", w=VW)
                nc.vector.tensor_tensor(
                    vv[:, :, 0:HD],
                    ps[:].rearrange("p (h e) -> p h e", e=HD),
                    vb_sb[:].rearrange("p (h w) -> p h w", w=VW)[:, :, 0:HD],
                    op=ADD)
                nc.vector.memset(vv[:, :, HD:HD + 1], 1.0)

            from functools import partial
            for mt in range(d.MT):
                for qn in range(nev):
                    chunks.append(partial(qk_chunk, mt, qn))
            for tt in range(TTH):
                chunks.append(partial(v_chunk, tt))
            return chunks

        def emit_qkv_half(hf):
            for c in qkv_chunks(hf):
                c()

        def emit_vm_r():
            wpv = wpt_d.rearrange("(j p) c -> j p c", p=P)
            for jt in range(d.JT):
                nc.sync.dma_start(wpt_sb[:, jt, :], wpv[jt])
            for h in range(d.HPC):
                ps = psum.tile([P, d.EV], F32, tag="acc", bufs=4,
                               name=f"vmps{h}")
                for kt in range(d.TT):
                    lhsT = v_sbs[kt // TTH][:, kt % TTH,
                                            h * VW:h * VW + HD]
                    nc.tensor.matmul(ps[0:HD, 0:1], lhsT, invT[:],
                                     start=(kt == 0), stop=(kt == d.TT - 1))
                pb = (h % 2) * HD
                nc.vector.tensor_copy(vm_sb[pb:pb + HD, h // 2:h // 2 + 1],
                                      ps[0:HD, 0:1])
            for cc in range(d.NCC):
                ps = psum.tile([P, d.CC], F32, tag="acc", bufs=4,
                               name=f"rps{cc}")
                for jt in range(d.JT):
                    nc.tensor.matmul(
                        ps[0:1, 0:d.CC], vm_sb[:, jt:jt + 1],
                        wpt_sb[:, jt, cc * d.CC:(cc + 1) * d.CC],
                        start=(jt == 0), stop=(jt == d.JT - 1))
                nc.vector.tensor_copy(
                    r_sb[0:1, cc * d.CC:(cc + 1) * d.CC], ps[0:1, 0:d.CC])

        def emit_attention(qc, fillers=()):
            fillers = list(fillers)
            per_head = -(-len(fillers) // d.HPC) if fillers else 0
            for h in range(d.HPC):
                hp, pb = h // 2, (h % 2) * HD
                kt_all = (qc + 1) * d.QCKT
                psy = [psum.tile([P, d.HALF], F32, tag="acc", bufs=4,
                                 name=f"psy{qc}_{h}_{i}")
                       for i in range(d.NHALF)]
                navp = [qc * d.QCKT + (half + 1) * d.HPP
                        for half in range(d.NHALF)]
                avdone = [0] * d.NHALF
                for kt in range(kt_all):
                    j = kt - qc * d.QCKT
                    col0 = max(0, j) * P
                    khf, kloc = kt // d.QCKT, (kt % d.QCKT) * P
                    st = psum.tile([P, d.QC], F32, tag="st",
                                   name=f"st{qc}_{h}_{kt}")
                    for half in range(d.NHALF):
                        a = max(col0, half * d.HALF)
                        b = (half + 1) * d.HALF
                        if a < b:
                            nc.tensor.matmul(
                                st[:, a:b],
                                kt_sbs[khf][pb:pb + HD, hp, kloc:kloc + P],
                                qt_sbs[qc][pb:pb + HD, hp, a:b],
                                start=True, stop=True)
                    pt = work.tile([P, d.QC], BF16, tag="pt", bufs=pt_bufs,
                                   name=f"pt{qc}_{h}_{kt}")
                    nc.scalar.activation(pt[:, col0:], st[:, col0:], Exp,
                                         bias=kbias_sb[:, kt:kt + 1],
                                         scale=0.125)
                    if j >= 0:
                        nc.vector.tensor_mul(pt[:, col0:col0 + P],
                                             pt[:, col0:col0 + P], triu_sb[:])
                    for half in range(d.NHALF):
                        a = max(col0, half * d.HALF)
                        b = (half + 1) * d.HALF
                        if a < b:
                            i = avdone[half]
                            nc.tensor.matmul(
                                psy[half][0:VW, a - half * d.HALF:],
                                v_sbs[khf][:, kt % d.QCKT, h * VW:(h + 1) * VW],
                                pt[:, a:b],
                                start=(i == 0), stop=(i == navp[half] - 1))
                            avdone[half] += 1
                for half in range(d.NHALF):
                    q0 = qc * d.QC + half * d.HALF
                    lq0 = half * d.HALF
                    dn = work.tile([1, d.HALF], F32, tag="dn", bufs=2,
                                   name=f"dn{qc}_{h}_{half}")
                    nc.vector.reciprocal(dn[0:1, :], psy[half][HD:HD + 1, :])
                    nc.vector.tensor_mul(dn[0:1, :], dn[0:1, :],
                                         mq_sb[0:1, q0:q0 + d.HALF])
                    rb = work.tile([P, d.HALF], F32, tag="rb", bufs=2,
                                   name=f"rb{qc}_{h}_{half}")
                    nc.gpsimd.partition_broadcast(rb[0:HD, :], dn[0:1, :])
                    nc.vector.tensor_mul(
                        yt_sbs[qc][pb:pb + HD, hp, lq0:lq0 + d.HALF],
                        psy[half][0:HD, :], rb[0:HD, :])
                for _ in range(per_head):
                    if fillers:
                        fillers.pop(0)()
            while fillers:
                fillers.pop(0)()

        def proj_chunks(qc):
            from functools import partial
            npj = d.QC // d.EV

            def p_chunk(ct, qn):
                q0 = qc * d.QC + qn * d.EV
                lq0 = qn * d.EV
                ps = psum.tile([P, d.EV], F32, tag="acc", bufs=4,
                               name=f"ops{qc}_{ct}_{qn}")
                for jt in range(d.JT):
                    nc.tensor.matmul(
                        ps[:], wpt_sb[:, jt, ct * P:(ct + 1) * P],
                        yt_sbs[qc][:, jt, lq0:lq0 + d.EV],
                        start=(jt == 0), stop=False)
                nc.tensor.matmul(ps[:], r_sb[0:1, ct * P:(ct + 1) * P],
                                 omq_sb[0:1, q0:q0 + d.EV],
                                 start=False, stop=True)
                osb = work.tile([P, d.EV], F32, tag="osb", bufs=2,
                                name=f"osb{qc}_{ct}_{qn}")
                nc.vector.tensor_scalar_add(osb[:], ps[:],
                                            bpc_sb[:, ct:ct + 1])
                nc.sync.dma_start(ot_v[ct][:, q0:q0 + d.EV], osb[:])

            return [partial(p_chunk, ct, qn)
                    for ct in range(d.CT) for qn in range(npj)]

        def emit_proj(qc):
            for c in proj_chunks(qc):
                c()

        emit_qkv_half(0)
        if d.NQC == 1:
            emit_vm_r()
            emit_attention(0)
            emit_proj(0)
        else:
            for hf in range(d.NQC - 1):
                emit_attention(hf, fillers=qkv_chunks(hf + 1))
            emit_vm_r()
            emit_attention(d.NQC - 1,
                           fillers=[c for qc in range(d.NQC - 1)
                                    for c in proj_chunks(qc)])
            emit_proj(d.NQC - 1)

    nc.compile()
    return nc


def prep_inputs(d: Dims, x, attn_mask, Wq, bq, Wk, bk, Wv, bv, Wp, bp):
    """Host-side shard + pack. Returns list of per-core input dicts."""
    f32 = np.float32
    per_core = []
    triu = np.triu(np.ones((P, P), dtype=f32)).astype(BF)
    for c in range(d.n_cores):
        b, g = c // d.gpb, c % d.gpb
        g0 = g * d.CG
        gsl = slice(g0, g0 + d.CG)
        xt = np.ascontiguousarray(x[b].T).astype(BF)
        wqt = np.ascontiguousarray(Wq[gsl, :].T).astype(BF)
        wkt = np.ascontiguousarray(Wk[gsl, :].T).astype(BF)
        wvt = np.ascontiguousarray(Wv[gsl, :].T).astype(BF)
        wpt = np.ascontiguousarray(Wp[:, gsl].T).astype(BF)
        qb = np.ascontiguousarray(
            bq[gsl].astype(f32).reshape(d.MT, P).T)
        kb = np.ascontiguousarray(
            bk[gsl].astype(f32).reshape(d.MT, P).T)
        vrow = np.zeros(d.HPC * VW, dtype=f32)
        for h in range(d.HPC):
            vrow[h * VW:h * VW + HD] = bv[g0 + h * HD:g0 + (h + 1) * HD]
        vb_t = np.ascontiguousarray(np.tile(vrow, (P, 1)))
        kbias = np.ascontiguousarray(
            np.where(attn_mask[b], 0.0, NEG_BIAS).astype(f32)
            .reshape(d.TT, P).T)
        bpc = np.ascontiguousarray(
            (bp if g == 0 else np.zeros_like(bp)).astype(f32)
            .reshape(d.CT, P).T)
        mq = attn_mask[b].astype(f32)[None, :]
        omq = (1.0 - mq).astype(BF)
        per_core.append({
            "xt": xt, "wqt": wqt, "wkt": wkt, "wvt": wvt, "wpt": wpt,
            "qb": qb, "kb": kb, "vb": vb_t, "kbias": kbias, "bpc": bpc,
            "mq": np.ascontiguousarray(mq),
            "omq": np.ascontiguousarray(omq),
            "triu": triu,
        })
    return per_core


def gather_output(d: Dims, results):
    out = np.empty((d.B, d.T, d.C), dtype=np.float32)
    for b in range(d.B):
        acc = results[b * d.gpb]["ot"].astype(np.float32)
        for g in range(1, d.gpb):
            acc = acc + results[b * d.gpb + g]["ot"]
        out[b] = acc.T
    return out


_CACHE = {}


def _get_program(d: Dims):
    key = (d.B, d.T, d.C, d.H, d.n_cores)
    if key not in _CACHE:
        _CACHE[key] = build_program(d)
    return _CACHE[key]


def kernel(x, attn_mask, Wq, bq, Wk, bk, Wv, bv, Wp, bp):
    from concourse.bass_utils import run_bass_kernel_spmd
    x = np.asarray(x)
    d = Dims(B=x.shape[0], T=x.shape[1], C=x.shape[2],
             H=16, n_cores=8)
    nc = _get_program(d)
    ins = prep_inputs(d, x, np.asarray(attn_mask),
                      np.asarray(Wq), np.asarray(bq), np.asarray(Wk),
                      np.asarray(bk), np.asarray(Wv), np.asarray(bv),
                      np.asarray(Wp), np.asarray(bp))
    res = run_bass_kernel_spmd(nc, ins, core_ids=list(range(d.n_cores)))
    return gather_output(d, res.results)
